# revision 35
# baseline (speedup 1.0000x reference)
"""Non-local block (B=4, C_in=256, C_int=128, C_out=256, N=T*H*W=4096) on 8
Trainium2 NeuronCores.

Sharding: data-parallel over batch (4 batches) x query-halves (2) = 8 cores.
Each core holds one batch's full x (for keys/values); the host rotates each
core's columns so its 2048 queries are always columns 0:2048 (attention is
permutation-invariant over keys). Per core: theta/phi/g projections, the
[2048q x 4096k] attention with softmax (keys on partitions), and the output
projection for its query half. Host gathers the 8 [256, 2048] slices.

Engine layout (~105.2-105.8us; prior 106.3/108.3us, baseline 109.8us):
  NOTE: the f32r ones-matmul + bf16->f32r cast in the epilogue is load-
  bearing: a bf16 ones stationary (with or without a mixed-dtype second
  accumulating matmul) produces corrupt partition sums on this hw.
  PE:   scores + y + projections + bf16 gT transposes (1 cyc/row vs 1.5
        for f32r) + denominator broadcast (ones matmul).  Single PSUM
        scope for the whole kernel (scores ring 2x[128,1024]=4 banks,
        y 1x[128,1024]=2, proj/piece ring 2x[128,512]=2) so there is no
        pool-boundary drain between the two query groups.  The y matmuls
        are DEFERRED one iteration: emitted at iter kb they would wait
        ~1us for exp(kb) and head-block the in-order PE queue, stalling
        the already-ready scores(kb+1) behind them -- scores paces the
        Act exp stream, so that head-block set the whole steady-state
        pace (worth ~2us).  scores(0) of each group is pre-emitted (in
        the prologue / at the previous group's kb30) so each exp stream
        starts with zero scores latency.  A matmul output cannot span 2
        PSUM banks, so scores/y stay as 512-free pairs, and standalone
        ldweights is broken for f32r, so each matmul reloads weights.
  Act:  exp stream + prologue/work-piece projection evacs + final-group
        y evac.  Exp table pre-loaded via a dummy exp during the DMA
        wait.  In the late, piece-free stretches of both groups every
        4th exp tile runs ENTIRELY on DVE as a Schraudolph approximation
        (Act skips the whole 1114ns tile plus its ~150ns of sequencer
        overhead; whole tiles beat column splits because the exp stream
        pays per-instruction overhead, not per-column) -- Act ~950 vs
        DVE ~990 per iteration, balanced.  Early group-1 odd (piece-
        free) kbs keep the [0:896]/[896:1024] column split.
  DVE:  softmax denominator: FULL-TILE adds only (sliced tensor_tensor
        ops lose the 2x hw mode), deferred one iteration so the queue
        never stalls; group-0 y evac (keeps the Act queue free between
        the two exp streams); Schraudolph approximate-exp share (one
        tensor_scalar: bf16 bits = round(s*184.665 + 16250.5) written
        via a uint16 bitcast -- adds <1e-3 to the output rms because
        softmax normalization cancels most of the per-weight error);
        epilogue cast/recip/scale.
  Pool (gpsimd): unused -- it cannot access PSUM and its software adds
        run at ~3.6us per [128,1024] bf16 tile.
  DMA:  three stages; stage-2 (wg/identity + x[512:1024]) gated on the
        warm-exp so the startup-critical stage-1 (wt/wp + x[0:512]) gets
        a bandwidth head start, and the x tail + cold constants gated
        behind the first theta evacuation.
"""

import sys
import types

import numpy as np

import concourse.bacc as bacc
import concourse.mybir as mybir
import concourse.tile as tile
from concourse.bass_utils import run_bass_kernel_spmd
from concourse.tile import add_dep_helper


def _install_ntff_hook():
    try:
        import antenv.axon_hooks  # noqa: F401
        return
    except ImportError:
        pass
    try:
        from trn_agent_boot.trn_boot import _ntff_profile_via_ctypes

        hook = _ntff_profile_via_ctypes("/opt/axon/libaxon_pjrt.so")
    except Exception:
        hook = None
    mod = types.ModuleType("antenv.axon_hooks")
    mod.get_axon_ntff_profile_hook = lambda: hook
    mod.set_axon_ntff_profile_hook = lambda h: None
    sys.modules["antenv.axon_hooks"] = mod


_install_ntff_hook()

F32 = mybir.dt.float32
F32R = mybir.dt.float32r
BF16 = mybir.dt.bfloat16
U16 = mybir.dt.uint16
AF = mybir.ActivationFunctionType
OP = mybir.AluOpType

P = 128
CI = 256  # input channels (2 chunks of 128)
CINT = 128  # intermediate channels
CO = 256  # output channels (2 blocks of 128)
N = 4096  # key/value positions (32 blocks of 128)
Q = 2048  # queries per core
B, T, H, W = 4, 4, 32, 32
NKB = N // P  # 32 key blocks

MM_DT = F32R

# Schraudolph bf16-bits exp: bits16 = round(s*184.665 + 16250.5)
SCHRAU_MUL = 184.66496
SCHRAU_ADD = 16250.5
# group-1 kb index where the Act/DVE exp split starts
SCHRAU_KB0 = 14
# column split of late group-1 exp tiles: Act [0:ESPL], DVE [ESPL:1024]
ESPL = 896


# cpak1 (stage-1): wtT 0:256 | wpT 256:512 | bt 512 | bp 513
CP1 = 514
# cpak2 (stage-2): wgT 0:256 | ident_bf 256:320 | bg 320 | bo 321:323 |
# ones_bf 323:387
CP2 = 387
# cpak_cold (stage-3): woT 0:256 | ones 256:384
CPC = 384


def build():
    nc = bacc.Bacc(None, target_bir_lowering=False, debug=False)

    xb = nc.dram_tensor("xb", [CI, N], F32, kind="ExternalInput").ap()
    cpak1 = nc.dram_tensor("cpak1", [P, CP1], F32, kind="ExternalInput").ap()
    cpak2 = nc.dram_tensor("cpak2", [P, CP2], F32, kind="ExternalInput").ap()
    cpakc = nc.dram_tensor("cpakc", [P, CPC], F32, kind="ExternalInput").ap()
    oq = nc.dram_tensor("oq", [CO, Q], F32, kind="ExternalOutput").ap()

    with tile.TileContext(nc) as tc:
        with (
            tc.tile_pool(name="big", bufs=1) as big,
            tc.tile_pool(name="tmp", bufs=6) as tmp,
            tc.tile_pool(name="ps_s", bufs=2, space="PSUM") as ps_s,
            tc.tile_pool(name="ps_y", bufs=1, space="PSUM") as ps_y,
            tc.tile_pool(name="ps_proj", bufs=2, space="PSUM") as ps_proj,
        ):
            # ---- Act exp-table preload (dummy exp on a zeroed column) ----
            warm = big.tile([P, 1], F32, tag="warm")
            warm2 = big.tile([P, 1], F32, tag="warm2")
            nc.vector.memset(warm[:], 0.0)

            # ---- stage-1 DMAs (startup-critical): theta/phi weights + x0
            cpak1_sb = big.tile([P, CP1], MM_DT, tag="cpak1")
            nc.sync.dma_start(cpak1_sb[:], cpak1.bitcast(MM_DT))
            cpak2_sb = big.tile([P, CP2], MM_DT, tag="cpak2")
            cpakc_sb = big.tile([P, CPC], MM_DT, tag="cpakc")

            x_sb = big.tile([P, 2, N], MM_DT, tag="x")
            xbr = xb.rearrange("(o p) n -> p o n", p=P).bitcast(MM_DT)
            nc.scalar.dma_start(x_sb[:, :, 0:512], xbr[:, :, 0:512])
            # table load + warm exp queue behind the x0 issue on Act queue
            warm_inst = nc.scalar.activation(
                out=warm2[:], in_=warm[:], func=AF.Exp
            )
            # ---- stage-2 DMAs, gated on the warm exp (~1.3us after the
            # stage-1 issues) so stage-1 gets a bandwidth head start
            stage2 = [
                nc.sync.dma_start(cpak2_sb[:], cpak2.bitcast(MM_DT)),
                nc.sync.dma_start(x_sb[:, :, 512:1024], xbr[:, :, 512:1024]),
            ]
            for d in stage2:
                add_dep_helper(d.ins, warm_inst.ins, sync=True,
                               reason="stage-2 after stage-1 head start")

            wtT = cpak1_sb[:, 0:256].rearrange("p (o c) -> p o c", o=2)
            wpT = cpak1_sb[:, 256:512].rearrange("p (o c) -> p o c", o=2)
            bt_sb = cpak1_sb[:, 512:513].bitcast(F32)
            bp_sb = cpak1_sb[:, 513:514].bitcast(F32)
            wgT = cpak2_sb[:, 0:256].rearrange("p (o c) -> p o c", o=2)
            ident_bf = cpak2_sb[:, 256:320].bitcast(BF16)
            bg_sb = cpak2_sb[:, 320:321].bitcast(F32)
            bo_sb = cpak2_sb[:, 321:323].bitcast(F32)
            ones_bf = cpak2_sb[:, 323:387].bitcast(BF16)
            woT = cpakc_sb[:, 0:256].rearrange("p (o c) -> p o c", o=2)
            ones_sb = cpakc_sb[:, 256:384]

            # SBUF buffers shared across phases
            theta_sb = big.tile([P, Q], MM_DT, tag="theta")
            phi_sb = big.tile([P, N], MM_DT, tag="phi")
            g_sb = big.tile([P, N], BF16, tag="g")
            gT_sb = big.tile([P, N], BF16, tag="gT")
            y_sb = big.tile([P, Q], MM_DT, tag="y")
            d_acc = big.tile([P, Q], BF16, tag="dacc")
            d_f32 = big.tile([P, Q], MM_DT, tag="df32")
            out_sb = big.tile([P, 2, Q], F32, tag="out")
            oqr = oq.rearrange("(o p) q -> p o q", p=P)

            # ---- projection emitters --------------------------------------
            def proj(which, j, on_act=False):
                wT, bias, dst = {
                    "t": (wtT, bt_sb, theta_sb),
                    "p": (wpT, bp_sb, phi_sb),
                    "g": (wgT, bg_sb, g_sb),
                }[which]
                sl = slice(j * 512, (j + 1) * 512)
                pp = ps_proj.tile([P, 512], F32, tag="pp", name=f"pp{which}{j}")
                nc.tensor.matmul(
                    pp[:], wT[:, 0, :], x_sb[:, 0, sl], start=True, stop=False
                )
                nc.tensor.matmul(
                    pp[:], wT[:, 1, :], x_sb[:, 1, sl], start=False, stop=True
                )
                if on_act:
                    return nc.scalar.activation(
                        out=dst[:, sl], in_=pp[:], func=AF.Identity,
                        bias=bias,
                    )
                return nc.vector.tensor_scalar(
                    out=dst[:, sl], in0=pp[:],
                    scalar1=bias, scalar2=None, op0=OP.add,
                )

            def gtq(j, on_act=False):
                # transpose g columns 512j..512j+512 (4 key blocks, bf16 at
                # 1 cycle/row) into a bf16 bitcast slice of one proj-ring
                # PSUM tile, then one 2x-mode copy into gT
                sl = slice(j * 512, (j + 1) * 512)
                pq = ps_proj.tile([P, 512], F32, tag="pp", name=f"pq{j}")
                pqb = pq[:, 0:256].bitcast(BF16)
                for k in range(4):
                    ksl = slice(j * 512 + k * P, j * 512 + (k + 1) * P)
                    nc.tensor.transpose(
                        pqb[:, k * P : (k + 1) * P], g_sb[:, ksl], ident_bf
                    )
                if on_act:
                    nc.scalar.activation(
                        out=gT_sb[:, sl], in_=pqb[:], func=AF.Copy
                    )
                else:
                    nc.vector.tensor_copy(out=gT_sb[:, sl], in_=pqb[:])

            # ---- prologue.  The PE queue is in-order, so scores(0) h0 is
            # emitted right after theta0/phi0 (before theta1/g0) and h1
            # right after theta1 -- the first exp can then start ~3us
            # earlier than if scores(0) sat behind the whole prologue.
            # g0's evac goes to DVE so it does not delay exp(0) on the Act
            # queue; gtq0 moves into the work list (still ahead of y(0)).
            gate_inst = proj("t", 0, on_act=True)
            proj("p", 0, on_act=True)
            s0_pre = ps_s.tile([P, 1024], F32, tag="s", name="s0_pre")
            nc.tensor.matmul(
                s0_pre[:, 0:512], phi_sb[:, 0:P], theta_sb[:, 0:512],
                start=True, stop=True,
            )
            # theta1 evac on DVE: the Act queue must reach exp(0,h0) with
            # nothing in front of it but the theta0/phi0 evacs
            proj("t", 1)
            nc.tensor.matmul(
                s0_pre[:, 512:1024], phi_sb[:, 0:P], theta_sb[:, 512:1024],
                start=True, stop=True,
            )

            def g0q0():
                # g0 + its transpose as one work piece at kb0: they must
                # stay ahead of y(0) in the in-order PE queue
                proj("g", 0)
                gtq(0)

            # ---- stage-3 DMAs: x tail + cold consts, gated on theta0 evac
            tail = [
                nc.sync.dma_start(x_sb[:, :, 1024:2048], xbr[:, :, 1024:2048]),
                nc.sync.dma_start(x_sb[:, :, 2048:3072], xbr[:, :, 2048:3072]),
                nc.sync.dma_start(x_sb[:, :, 3072:4096], xbr[:, :, 3072:4096]),
                nc.sync.dma_start(cpakc_sb[:], cpakc.bitcast(MM_DT)),
            ]
            for d in tail:
                add_dep_helper(d.ins, gate_inst.ins, sync=True,
                               reason="x tail after startup-critical DMAs")

            # deferred projection pieces, one per kb of group 0.  phi j
            # feeds scores kb=4j (emitted one kb early); gT quad j feeds y
            # kb=4j; theta j2/j3 feed group 1.
            work = [
                (g0q0, ()),
                (proj, ("p", 1)), (proj, ("g", 1)), (proj, ("t", 2)),
                (gtq, (1,)),
            ]
            for j in range(2, 8):
                work += [(proj, ("p", j)), (proj, ("g", j)), (gtq, (j,))]
            work += [(proj, ("t", 3))]

            def exp_pieces(gi, kb, final):
                """[(engine, c0, c1)] for the exp of tile (gi, kb)."""
                if gi == 0:
                    if kb < 2:
                        return [("act", 0, 512), ("act", 512, 1024)]
                    if kb >= 23:
                        # same rotation as late group 1: two tiles entirely
                        # on DVE (kb 25/29 -- clear of the kb22-23 work-list
                        # tail where DVE already has the t3 evacuation)
                        if kb in (25, 29):
                            return [("dve", 0, 1024)]
                        return [("act", 0, 1024)]
                    return [("act", 0, 1024)]
                if final and kb >= NKB - 2:
                    return [("act", 0, 512), ("act", 512, 1024)]
                if kb >= SCHRAU_KB0:
                    # every 4th tile entirely on DVE: Act skips it (a full
                    # 1114ns + instruction overhead), balancing Act ~950 vs
                    # DVE ~990 per iteration
                    if kb % 4 == 3:
                        return [("dve", 0, 1024)]
                    return [("act", 0, 1024)]
                if kb >= 3 and kb % 2 == 1:
                    return [("act", 0, ESPL), ("dve", ESPL, 1024)]
                return [("act", 0, 1024)]

            def attn_group(gi, q0, qw, work=None, pending_out=None,
                           final=False, s_pre=None, s_pre2=None,
                           tail_work=None):
                qsl = slice(q0, q0 + qw)
                nh = qw // 512
                with nc.named_scope(f"attn{gi}"):
                    y_ps = ps_y.tile([P, qw], F32, tag="y", name=f"y_ps{gi}")

                    def scores(kb):
                        s_ps = ps_s.tile(
                            [P, qw], F32, tag="s", name=f"s{gi}_{kb}"
                        )
                        for h in range(nh):
                            nc.tensor.matmul(
                                s_ps[:, h * 512 : (h + 1) * 512],
                                phi_sb[:, kb * P : (kb + 1) * P],
                                theta_sb[:, q0 + h * 512 : q0 + (h + 1) * 512],
                                start=True, stop=True,
                            )
                        return s_ps

                    s_cur = s_pre if s_pre is not None else scores(0)
                    pend_at = [None]
                    pend_y = [None]

                    def emit_y(a, akb):
                        for h in range(nh):
                            hsl = slice(h * 512, (h + 1) * 512)
                            nc.tensor.matmul(
                                y_ps[:, hsl],
                                gT_sb[:, akb * P : (akb + 1) * P],
                                a[:, hsl],
                                start=akb == 0, stop=akb == NKB - 1,
                            )

                    for kb in range(NKB):
                        at = tmp.tile(
                            [P, qw], BF16, tag="attn", name=f"at{gi}_{kb}"
                        )
                        for eng, c0, c1 in exp_pieces(gi, kb, final):
                            if eng == "act":
                                nc.scalar.activation(
                                    out=at[:, c0:c1], in_=s_cur[:, c0:c1],
                                    func=AF.Exp,
                                )
                            else:
                                nc.vector.tensor_scalar(
                                    out=at[:, c0:c1].bitcast(U16),
                                    in0=s_cur[:, c0:c1],
                                    scalar1=SCHRAU_MUL, scalar2=SCHRAU_ADD,
                                    op0=OP.mult, op1=OP.add,
                                )
                        if kb + 1 < NKB:
                            # feed the PE the next scores before y(kb) so it
                            # is not idle while exp(kb) runs
                            if kb == 0 and s_pre2 is not None:
                                s_cur = s_pre2
                            else:
                                s_cur = scores(kb + 1)
                        # one deferred projection piece per kb (group 0)
                        if work:
                            fn, arg = work.pop(0)
                            fn(*arg)
                        # y matmuls deferred ONE iteration: emitted at iter
                        # kb they would wait ~1us for exp(kb) and head-block
                        # the in-order PE queue (stalling the already-ready
                        # scores(kb+1) behind them, which paces the Act exp
                        # stream).  At iter kb+1 the at tile is settled and
                        # every PE op is ready when the queue reaches it.
                        if pend_y[0] is not None:
                            emit_y(*pend_y[0])
                        pend_y[0] = (at, kb)
                        if final and kb == NKB - 1:
                            emit_y(*pend_y[0])
                            pend_y[0] = None
                        # softmax denominator: full-tile DVE adds (the 2x
                        # hw mode engages only on full-tile ops), DEFERRED
                        # one iteration so the DVE queue never stalls: at
                        # iter kb it accumulates at(kb-1), which Act/DVE
                        # finished writing last iteration.  Final kb runs
                        # immediately, split per half, so the h0 epilogue
                        # overlaps the h1 exp.
                        def dacc(a, c0, c1):
                            dsl = slice(q0 + c0, q0 + c1)
                            if a[1]:
                                nc.vector.tensor_copy(
                                    out=d_acc[:, dsl], in_=a[0][:, c0:c1]
                                )
                            else:
                                nc.vector.tensor_tensor(
                                    out=d_acc[:, dsl], in0=d_acc[:, dsl],
                                    in1=a[0][:, c0:c1], op=OP.add,
                                )

                        if pend_at[0] is not None:
                            dacc(pend_at[0], 0, qw)
                        pend_at[0] = (at, kb == 0)
                        if final and kb == NKB - 1:
                            dacc(pend_at[0], 0, 512)
                            dacc(pend_at[0], 512, 1024)
                            pend_at[0] = None
                        # previous group's output projection, one piece per
                        # kb starting at kb=2
                        if pending_out and kb >= 2 and kb % 2 == 0:
                            pending_out.pop(0)()
                        # pre-emit the NEXT group's first scores tile here so
                        # its exp can follow this group's last exp seamlessly
                        if tail_work and kb == NKB - 2:
                            tail_work.pop(0)()
                    if pend_y[0] is not None:
                        emit_y(*pend_y[0])
                        pend_y[0] = None
                    if pend_at[0] is not None:
                        dacc(pend_at[0], 0, qw)
                        pend_at[0] = None
                    while pending_out:
                        pending_out.pop(0)()
                    # evacuate y: DVE for group 0 (keeps the Act queue free
                    # between the two exp streams); Act for the final group
                    for h in range(nh):
                        hsl = slice(h * 512, (h + 1) * 512)
                        qhsl = slice(q0 + h * 512, q0 + (h + 1) * 512)
                        if final:
                            nc.scalar.activation(
                                out=y_sb[:, qhsl], in_=y_ps[:, hsl],
                                func=AF.Copy,
                            )
                        else:
                            nc.vector.tensor_copy(
                                out=y_sb[:, qhsl], in_=y_ps[:, hsl]
                            )

                pieces = []
                rd = big.tile([P, qw], F32, tag=f"rd{gi}")

                def cast_h(h):
                    qhsl = slice(q0 + h * 512, q0 + (h + 1) * 512)
                    nc.vector.tensor_copy(
                        out=d_f32[:, qhsl], in_=d_acc[:, qhsl]
                    )

                def dbc_recip_h(h):
                    # ones-matmul partition-reduce+broadcast, reciprocal.
                    # The bf16->f32r cast runs earlier (at the group
                    # boundary for the non-final group) so the d_bc matmul
                    # never head-blocks the next group's PE queue.
                    hsl = slice(h * 512, (h + 1) * 512)
                    qhsl = slice(q0 + h * 512, q0 + (h + 1) * 512)
                    with nc.named_scope(f"epi{gi}"):
                        d_bc = ps_proj.tile(
                            [P, 512], F32, tag="pp", name=f"dbc{gi}{h}"
                        )
                        nc.tensor.matmul(
                            d_bc[:], ones_sb, d_f32[:, qhsl],
                            start=True, stop=True,
                        )
                        nc.vector.reciprocal_approx_fast(
                            out=rd[:, hsl], in_=d_bc[:],
                        )

                def epi_h(h):
                    cast_h(h)
                    dbc_recip_h(h)

                def out_piece(blk, h):
                    hsl = slice(h * 512, (h + 1) * 512)
                    qhsl = slice(q0 + h * 512, q0 + (h + 1) * 512)
                    with nc.named_scope(f"outp{gi}"):
                        po = ps_proj.tile(
                            [P, 512], F32, tag="pp", name=f"po{gi}{blk}{h}"
                        )
                        nc.tensor.matmul(
                            po[:], woT[:, blk, :], y_sb[:, qhsl],
                            start=True, stop=True,
                        )
                        # out = (po + b_out) * (1/d) in one DVE pass
                        nc.vector.scalar_tensor_tensor(
                            out=out_sb[:, blk, qhsl], in0=po[:],
                            scalar=bo_sb[:, blk : blk + 1], in1=rd[:, hsl],
                            op0=OP.add, op1=OP.mult,
                        )
                        nc.sync.dma_start(
                            oqr[:, blk, qhsl], out_sb[:, blk, qhsl]
                        )

                if final:
                    # all four out-proj matmuls first, back-to-back into the
                    # now-dead scores-ring PSUM tiles (they need only the y
                    # evacs): the PE stays warm and the DVE epilogue chain
                    # (cast/recip/scale) overlaps them instead of
                    # serializing through cold-p-state matmuls
                    pts = []
                    for h in range(nh):
                        qhsl = slice(q0 + h * 512, q0 + (h + 1) * 512)
                        pt = ps_s.tile([P, qw], F32, tag="s", name=f"pot{h}")
                        for blk in range(2):
                            nc.tensor.matmul(
                                pt[:, blk * 512 : (blk + 1) * 512],
                                woT[:, blk, :], y_sb[:, qhsl],
                                start=True, stop=True,
                            )
                        pts.append(pt)
                    for h in range(nh):
                        hsl = slice(h * 512, (h + 1) * 512)
                        qhsl = slice(q0 + h * 512, q0 + (h + 1) * 512)
                        epi_h(h)
                        with nc.named_scope(f"outp{gi}"):
                            for blk in range(2):
                                nc.vector.scalar_tensor_tensor(
                                    out=out_sb[:, blk, qhsl],
                                    in0=pts[h][:, blk * 512 : (blk + 1) * 512],
                                    scalar=bo_sb[:, blk : blk + 1],
                                    in1=rd[:, hsl],
                                    op0=OP.add, op1=OP.mult,
                                )
                                nc.sync.dma_start(
                                    oqr[:, blk, qhsl], out_sb[:, blk, qhsl]
                                )
                    return []
                # non-final: casts now (DVE-only, d_acc is complete);
                # dbc/recip + out-proj pieces interleave into the next group
                for h in range(nh):
                    cast_h(h)
                for h in range(nh):
                    pieces.append(lambda h=h: dbc_recip_h(h))
                for h in range(nh):
                    for blk in range(2):
                        pieces.append(lambda blk=blk, h=h: out_piece(blk, h))
                return pieces

            s1_holder = []

            def make_spre(kb, name):
                s1 = ps_s.tile([P, 1024], F32, tag="s", name=name)
                for h in range(2):
                    nc.tensor.matmul(
                        s1[:, h * 512 : (h + 1) * 512],
                        phi_sb[:, kb * P : (kb + 1) * P],
                        theta_sb[:, 1024 + h * 512 : 1024 + (h + 1) * 512],
                        start=True, stop=True,
                    )
                s1_holder.append(s1)

            out0 = attn_group(0, 0, 1024, work=work, s_pre=s0_pre,
                              tail_work=[lambda: make_spre(0, "s1_pre")])
            # scores(B,1) pre-emitted at the boundary (right after group
            # 0's trailing y flush) so exp(B,1) follows exp(B,0) seamlessly
            make_spre(1, "s2_pre")
            attn_group(1, 1024, 1024, pending_out=out0, final=True,
                       s_pre=s1_holder[0], s_pre2=s1_holder[1])

    nc.compile()
    return nc


_NC_CACHE = None
LAST_EXEC_TIME_NS = None
LAST_TRACE = None
LAST_RESULTS = None


def _get_nc():
    global _NC_CACHE
    if _NC_CACHE is None:
        _NC_CACHE = build()
    return _NC_CACHE


def kernel(**inputs):
    x = np.ascontiguousarray(np.asarray(inputs["x"], dtype=np.float32))
    assert x.shape == (B, CI, T, H, W), x.shape
    xf = x.reshape(B, CI, N)
    w = {
        k: np.ascontiguousarray(np.asarray(inputs[k], dtype=np.float32))
        for k in (
            "w_theta", "b_theta", "w_phi", "b_phi", "w_g", "b_g", "w_out",
            "b_out",
        )
    }

    def proj_t(wm):
        # [p, o*128+c] = wm[c, o*128+p]
        return wm.T.reshape(2, P, P).transpose(1, 0, 2).reshape(P, 2 * P)

    woT_h = w["w_out"].reshape(2, P, CINT).transpose(2, 0, 1).reshape(P, 2 * P)
    # bf16 identity packed into 64 f32 columns
    i16 = np.zeros((P, P), np.uint32)
    i16[np.arange(P), np.arange(P)] = 0x3F80
    ident_bf_packed = (i16[:, 0::2] | (i16[:, 1::2] << 16)).view(np.float32)
    CPAK1 = np.ascontiguousarray(
        np.concatenate(
            [
                proj_t(w["w_theta"]), proj_t(w["w_phi"]),
                w["b_theta"][:, None], w["b_phi"][:, None],
            ],
            axis=1,
        )
    )
    assert CPAK1.shape == (P, CP1), CPAK1.shape
    ones_bf_packed = np.full((P, P // 2), 0x3F803F80, np.uint32).view(
        np.float32
    )
    CPAK2 = np.ascontiguousarray(
        np.concatenate(
            [
                proj_t(w["w_g"]), ident_bf_packed,
                w["b_g"][:, None],
                w["b_out"][:P, None], w["b_out"][P:, None],
                ones_bf_packed,
            ],
            axis=1,
        )
    )
    assert CPAK2.shape == (P, CP2), CPAK2.shape
    CPAKC = np.ascontiguousarray(
        np.concatenate([woT_h, np.ones((P, P), np.float32)], axis=1)
    )
    assert CPAKC.shape == (P, CPC), CPAKC.shape

    in_maps = []
    for core in range(8):
        b, h = core // 2, core % 2
        if h == 0:
            xcore = xf[b]
        else:
            xcore = np.ascontiguousarray(
                np.concatenate([xf[b][:, Q:], xf[b][:, :Q]], axis=1)
            )
        in_maps.append(
            {"xb": xcore, "cpak1": CPAK1, "cpak2": CPAK2, "cpakc": CPAKC}
        )

    nc = _get_nc()
    res = run_bass_kernel_spmd(nc, in_maps, core_ids=list(range(8)))
    global LAST_EXEC_TIME_NS, LAST_TRACE, LAST_RESULTS
    LAST_EXEC_TIME_NS = res.exec_time_ns
    LAST_TRACE = (
        res.instructions_and_trace[1] if res.instructions_and_trace else None
    )
    LAST_RESULTS = res

    out = np.empty((B, CO, N), np.float32)
    for core in range(8):
        b, h = core // 2, core % 2
        out[b][:, h * Q : (h + 1) * Q] = res.results[core]["oq"]
    return out.reshape(B, CO, T, H, W)
